# revision 18
# baseline (speedup 1.0000x reference)
"""Trainium2 Bass kernel for nn_Attention_67156108640667 (pooling attention).

reference:
    energies = einsum('btd,d->bt', x, v)         # GEMV per batch
    weights  = softmax(energies, axis=1)          # [B, T]
    context  = einsum('bt,btd->bd', weights, x)   # weighted-sum pool

Shapes: B=64, T=4096, D=512, f32.  Data-parallel over B across 8 cores.

Per core (8 batches, single HBM pass, t-mapping t = p*32 + j so each
partition reads 16 KB-contiguous spans):
  - x streams in 2 MiB chunks alternating between the two HWDGE rings.
  - per [128, 512] tile either (a) fused DVE scalar_tensor_tensor:
    prod = x*v (bf16, kept for context) + fp32 energy column, or
  - (b) PE route: 4x transpose -> xT, fp32 matmul against v -> energy
    column; DVE only casts x->bf16 for the context matmul.
  - softmax per batch on the [128, 32] energy matrix (PE for the
    cross-partition max/sum broadcasts, ACT exp with fp32 accumulate).
  - context via PE bf16 matmuls in two PSUM accumulators (prod-tiles need
    a final /v, x-tiles don't), epilogue scales on ACT/GpSimd.
"""

from contextlib import ExitStack

import numpy as np

import concourse.bass as bass
import concourse.bacc as bacc
import concourse.tile as tile
from concourse import mybir
from concourse import bass_utils

B, T, D = 64, 4096, 512
NCORES = 8
BPC = B // NCORES            # 8 batches per core
P = 128                      # SBUF partitions
NT = T // P                  # 32 t-tiles per batch
CHUNK_TILES = 8              # t-tiles per DMA chunk (8*128*512*4B = 2 MiB)
NCHUNK = NT // CHUNK_TILES   # 4 chunks per batch
OFF_PER_CHUNK = 3            # tiles per chunk computed via the PE route
DQ = D // P                  # 4 d-blocks of 128

F32 = mybir.dt.float32
BF16 = mybir.dt.bfloat16
ALU = mybir.AluOpType
ACTF = mybir.ActivationFunctionType
AX = mybir.AxisListType


def build_tile_kernel(tc, x_ap, vb_ap, vinv_ap, vcol_ap, id_ap, ctx_ap, w_ap):
    """Emit the per-core program.

    x_ap:    [BPC, T, D] f32 DRAM in  (t-major)
    vb_ap:   [P, D]      f32 DRAM in  (v row broadcast to 128 partitions)
    vinv_ap: [1, D]      f32 DRAM in  (1/v row)
    vcol_ap: [DQ, P]     f32 DRAM in  (v as DQ blocks of 128)
    id_ap:   [P, P]      f32 DRAM in  (identity)
    ctx_ap:  [BPC, D]    f32 DRAM out
    w_ap:    [BPC, T]    f32 DRAM out
    """
    nc = tc.nc
    with ExitStack() as ctx:
        consts = ctx.enter_context(tc.tile_pool(name="consts", bufs=1))
        xpool = ctx.enter_context(tc.tile_pool(name="xpool", bufs=4))
        prodp = ctx.enter_context(tc.tile_pool(name="prodp", bufs=3))
        statp = ctx.enter_context(tc.tile_pool(name="statp", bufs=3))
        xtp = ctx.enter_context(tc.tile_pool(name="xtp", bufs=2))
        outp = ctx.enter_context(tc.tile_pool(name="outp", bufs=2))
        ps_small = ctx.enter_context(tc.tile_pool(name="ps_small", bufs=3, space="PSUM"))
        ps_xt = ctx.enter_context(tc.tile_pool(name="ps_xt", bufs=2, space="PSUM"))
        ps_ctx = ctx.enter_context(tc.tile_pool(name="ps_ctx", bufs=3, space="PSUM"))

        # ---- constants (all DMA'd from host) ----
        v_bcast = consts.tile([P, D], F32)
        nc.sync.dma_start(out=v_bcast, in_=vb_ap)
        v_inv = consts.tile([1, D], F32)
        nc.sync.dma_start(out=v_inv, in_=vinv_ap)
        v_cols = consts.tile([P, DQ], F32)
        nc.sync.dma_start(out=v_cols, in_=vcol_ap)
        ident = consts.tile([P, P], F32)
        nc.sync.dma_start(out=ident, in_=id_ap)
        ones_row = consts.tile([1, P], F32)
        nc.vector.memset(ones_row, 1.0)
        neg_ones_row = consts.tile([1, P], F32)
        nc.vector.memset(neg_ones_row, -1.0)
        ones_col = consts.tile([P, 1], F32)
        nc.vector.memset(ones_col, 1.0)

        # x[b] viewed so partition p holds rows p*NT .. p*NT+NT-1
        for b in range(BPC):
            xb = x_ap[b].rearrange("(p j) d -> p j d", p=P)  # [P, NT, D]

            # ---- stage 1: stream x; energies per tile ----
            E_cols = statp.tile([P, NT], F32, tag="E_cols")
            prod_b = prodp.tile([P, NT, D], BF16, tag="prod")
            for c in range(NCHUNK):
                xc = xpool.tile([P, CHUNK_TILES, D], F32, tag="xc")
                dma_eng = nc.sync if (b * NCHUNK + c) % 2 == 0 else nc.scalar
                dma_eng.dma_start(
                    out=xc, in_=xb[:, c * CHUNK_TILES : (c + 1) * CHUNK_TILES, :]
                )
                for jj in range(CHUNK_TILES):
                    j = c * CHUNK_TILES + jj
                    if jj < CHUNK_TILES - OFF_PER_CHUNK:
                        # (a) fused DVE route: prod (bf16) + energy column
                        nc.vector.scalar_tensor_tensor(
                            out=prod_b[:, j, :],
                            in0=xc[:, jj, :],
                            scalar=1.0,
                            in1=v_bcast,
                            op0=ALU.mult,
                            op1=ALU.mult,
                            accum_out=E_cols[:, j : j + 1],
                        )
                    else:
                        # (b) PE route: transpose 4 d-blocks, matvec vs v
                        xt_ps = ps_xt.tile([P, D], F32, tag="xt_ps")
                        for q in range(DQ):
                            nc.tensor.transpose(
                                xt_ps[:, q * P : (q + 1) * P],
                                xc[:, jj, q * P : (q + 1) * P],
                                ident,
                            )
                        xt_sb = xtp.tile([P, D], F32, tag="xt_sb")
                        nc.scalar.copy(xt_sb, xt_ps)
                        e_ps = ps_small.tile([P, 1], F32, tag="ps_small")
                        for q in range(DQ):
                            nc.tensor.matmul(
                                e_ps,
                                lhsT=xt_sb[:, q * P : (q + 1) * P],
                                rhs=v_cols[:, q : q + 1],
                                start=(q == 0),
                                stop=(q == DQ - 1),
                                skip_group_check=True,
                            )
                        nc.scalar.copy(E_cols[:, j : j + 1], e_ps)
                        # bf16 copy of x for the context matmul
                        nc.vector.tensor_copy(prod_b[:, j, :], xc[:, jj, :])

            # ---- stage 2: softmax over the [P, NT] energy matrix ----
            m1 = statp.tile([P, 1], F32, tag="m1")
            nc.vector.tensor_reduce(m1, E_cols, axis=AX.X, op=ALU.max)
            mT_ps = ps_small.tile([1, P], F32, tag="ps_small")
            nc.tensor.transpose(mT_ps, m1, ident)
            mT = statp.tile([1, P], F32, tag="mT")
            nc.scalar.copy(mT, mT_ps)
            m = statp.tile([1, 1], F32, tag="m")
            nc.vector.tensor_reduce(m, mT, axis=AX.X, op=ALU.max)
            negm_ps = ps_small.tile([P, 1], F32, tag="ps_small")
            nc.tensor.matmul(negm_ps, lhsT=neg_ones_row, rhs=m, start=True, stop=True)
            negm = statp.tile([P, 1], F32, tag="negm")
            nc.scalar.copy(negm, negm_ps)
            P_cols = statp.tile([P, NT], BF16, tag="P_cols")
            s1 = statp.tile([P, 1], F32, tag="s1")
            nc.scalar.activation(
                P_cols, E_cols, ACTF.Exp, bias=negm, scale=1.0, accum_out=s1
            )

            # ---- stage 3: two context accumulators (prod-route / x-route) --
            js_a = [j for j in range(NT) if (j % CHUNK_TILES) < CHUNK_TILES - OFF_PER_CHUNK]
            js_b = [j for j in range(NT) if (j % CHUNK_TILES) >= CHUNK_TILES - OFF_PER_CHUNK]
            ctx_pa = ps_ctx.tile([1, D], F32, tag="ctx_ps", name="ctx_pa")
            ctx_pb = (
                ps_ctx.tile([1, D], F32, tag="ctx_ps", name="ctx_pb") if js_b else None
            )
            for acc, js in ((ctx_pa, js_a), (ctx_pb, js_b)):
                for idx, j in enumerate(js):
                    nc.tensor.matmul(
                        acc,
                        lhsT=P_cols[:, j : j + 1],
                        rhs=prod_b[:, j, :],
                        start=(idx == 0),
                        stop=(idx == len(js) - 1),
                        skip_group_check=True,
                    )

            # total sum across partitions, reciprocal, broadcast to partitions
            s_ps = ps_small.tile([1, 1], F32, tag="ps_small")
            nc.tensor.matmul(s_ps, lhsT=s1, rhs=ones_col, start=True, stop=True)
            s_sb = statp.tile([1, 1], F32, tag="s_sb")
            nc.scalar.copy(s_sb, s_ps)
            r = statp.tile([1, 1], F32, tag="r")
            nc.vector.reciprocal(r, s_sb)
            r_ps = ps_small.tile([P, 1], F32, tag="ps_small")
            nc.tensor.matmul(r_ps, lhsT=ones_row, rhs=r, start=True, stop=True)
            r_col = statp.tile([P, 1], F32, tag="r_col")
            nc.scalar.copy(r_col, r_ps)

            # weights output: w[b, p*NT + j] = P_cols[p, j] * r  (no transpose)
            w_sb = outp.tile([P, NT], F32, tag="w_sb")
            nc.scalar.activation(w_sb, P_cols, ACTF.Copy, bias=0.0, scale=r_col)
            nc.sync.dma_start(out=w_ap[b].rearrange("(p j) -> p j", p=P), in_=w_sb)

            # context epilogue: (ctx_a * 1/v + ctx_b) * r
            ctx_a = outp.tile([1, D], F32, tag="ctx_a")
            nc.scalar.activation(ctx_a, ctx_pa, ACTF.Copy, bias=0.0, scale=r)
            ctx_sb = outp.tile([1, D], F32, tag="ctx_sb")
            nc.gpsimd.tensor_mul(ctx_sb, ctx_a, v_inv)
            if ctx_pb is not None:
                ctx_b = outp.tile([1, D], F32, tag="ctx_b")
                nc.scalar.activation(ctx_b, ctx_pb, ACTF.Copy, bias=0.0, scale=r)
                ctx_f = outp.tile([1, D], F32, tag="ctx_f")
                nc.gpsimd.tensor_add(ctx_f, ctx_sb, ctx_b)
            else:
                ctx_f = ctx_sb
            nc.sync.dma_start(out=ctx_ap[b : b + 1, :], in_=ctx_f)


_CACHED_NC = None


def _get_nc():
    global _CACHED_NC
    if _CACHED_NC is not None:
        return _CACHED_NC
    nc = bacc.Bacc(
        "TRN2",
        target_bir_lowering=False,
        debug=False,
        enable_asserts=False,
        num_devices=NCORES,
    )
    x = nc.dram_tensor("x", [BPC, T, D], F32, kind="ExternalInput")
    vb = nc.dram_tensor("vb", [P, D], F32, kind="ExternalInput")
    vinv = nc.dram_tensor("vinv", [1, D], F32, kind="ExternalInput")
    vcol = nc.dram_tensor("vcol", [P, DQ], F32, kind="ExternalInput")
    idm = nc.dram_tensor("idm", [P, P], F32, kind="ExternalInput")
    ctx_out = nc.dram_tensor("ctx", [BPC, D], F32, kind="ExternalOutput")
    w_out = nc.dram_tensor("w", [BPC, T], F32, kind="ExternalOutput")
    with tile.TileContext(nc) as tc:
        build_tile_kernel(
            tc, x.ap(), vb.ap(), vinv.ap(), vcol.ap(), idm.ap(), ctx_out.ap(), w_out.ap()
        )
    nc.compile()
    _CACHED_NC = nc
    return nc


def _make_const_inputs(v):
    vrow = v[:, 0].astype(np.float32)                     # [D]
    vb = np.broadcast_to(vrow, (P, D)).copy()
    vinv = (1.0 / vrow)[None, :].copy()
    vcol = np.ascontiguousarray(vrow.reshape(DQ, P).T)    # [P, DQ]
    idm = np.eye(P, dtype=np.float32)
    return vb, vinv, vcol, idm


def _unpermute_w(w_core):
    # device layout: w[b, p*NT + j] for tile-col j, partition p; t = p*NT + j
    # which is exactly t-major already (p*NT + j spans 0..T-1 in order)
    return w_core


def _run(encoder_outputs, attn_weights_param, trace=False, **kw):
    nc = _get_nc()
    x = np.ascontiguousarray(np.asarray(encoder_outputs, dtype=np.float32))
    v = np.ascontiguousarray(np.asarray(attn_weights_param, dtype=np.float32))
    vb, vinv, vcol, idm = _make_const_inputs(v)
    in_maps = [
        {
            "x": x[c * BPC : (c + 1) * BPC],
            "vb": vb,
            "vinv": vinv,
            "vcol": vcol,
            "idm": idm,
        }
        for c in range(NCORES)
    ]
    res = bass_utils.run_bass_kernel_spmd(
        nc, in_maps, core_ids=list(range(NCORES)), trace=trace, **kw
    )
    context = np.concatenate([res.results[c]["ctx"] for c in range(NCORES)], axis=0)
    weights = np.concatenate(
        [_unpermute_w(res.results[c]["w"]) for c in range(NCORES)], axis=0
    )
    return (context, weights), res


def kernel(encoder_outputs, attn_weights_param):
    (context, weights), _ = _run(encoder_outputs, attn_weights_param, trace=False)
    return (context, weights)


# revision 19
# speedup vs baseline: 1.6612x; 1.6612x over previous
"""Trainium2 Bass kernel for nn_Attention_67156108640667 (pooling attention).

reference:
    energies = einsum('btd,d->bt', x, v)         # GEMV per batch
    weights  = softmax(energies, axis=1)          # [B, T]
    context  = einsum('bt,btd->bd', weights, x)   # weighted-sum pool

Shapes: B=64, T=4096, D=512, f32.  Data-parallel over B across 8 cores.

Per core (8 batches, single HBM pass, t-mapping t = p*32 + j so each
partition reads 16 KB-contiguous spans):
  - x streams in 2 MiB chunks via SWDGE DMAs that cast f32 -> fp16 in
    flight (HBM reads stay f32; SBUF gets fp16).
  - per [128, 512] tile one fused DVE scalar_tensor_tensor in 2x mode:
    prod = x*v (fp16, kept for the context matmul) + exact fp32 energy
    column via the instruction accumulator.
  - softmax per batch on the [128, 32] energy matrix (PE for the
    cross-partition max/sum broadcasts, ACT exp with fp32 accumulate).
  - context = sum_t exp[t] * prod[t, :] via PE fp16 matmuls in PSUM;
    epilogue scales by 1/sum (ACT) and 1/v (GpSimd).
"""

from contextlib import ExitStack

import numpy as np

import concourse.bass as bass
import concourse.bacc as bacc
import concourse.tile as tile
from concourse import mybir
from concourse import bass_utils

B, T, D = 64, 4096, 512
NCORES = 8
BPC = B // NCORES            # 8 batches per core
P = 128                      # SBUF partitions
NT = T // P                  # 32 t-tiles per batch
CHUNK_TILES = 8              # t-tiles per DMA chunk (2 MiB of f32 reads)
NCHUNK = NT // CHUNK_TILES   # 4 chunks per batch

F32 = mybir.dt.float32
FP16 = mybir.dt.float16
ALU = mybir.AluOpType
ACTF = mybir.ActivationFunctionType
AX = mybir.AxisListType


def build_tile_kernel(tc, x_ap, vb_ap, vinv_ap, id_ap, ctx_ap, w_ap):
    """Emit the per-core program.

    x_ap:    [BPC, T, D] f32 DRAM in  (t-major)
    vb_ap:   [P, D]      fp16 DRAM in (v row broadcast to 128 partitions)
    vinv_ap: [1, D]      f32 DRAM in  (1/v row)
    id_ap:   [P, P]      f32 DRAM in  (identity)
    ctx_ap:  [BPC, D]    f32 DRAM out
    w_ap:    [BPC, T]    f32 DRAM out
    """
    nc = tc.nc
    with ExitStack() as ctx:
        consts = ctx.enter_context(tc.tile_pool(name="consts", bufs=1))
        xpool = ctx.enter_context(tc.tile_pool(name="xpool", bufs=8))
        prodp = ctx.enter_context(tc.tile_pool(name="prodp", bufs=3))
        statp = ctx.enter_context(tc.tile_pool(name="statp", bufs=3))
        outp = ctx.enter_context(tc.tile_pool(name="outp", bufs=3))
        ps_small = ctx.enter_context(tc.tile_pool(name="ps_small", bufs=4, space="PSUM"))
        ps_ctx = ctx.enter_context(tc.tile_pool(name="ps_ctx", bufs=3, space="PSUM"))

        # ---- constants (DMA'd from host) ----
        v_bcast = consts.tile([P, D], FP16)
        nc.sync.dma_start(out=v_bcast, in_=vb_ap)
        v_inv = consts.tile([1, D], F32)
        nc.sync.dma_start(out=v_inv, in_=vinv_ap)
        ident = consts.tile([P, P], F32)
        nc.sync.dma_start(out=ident, in_=id_ap)
        ones_row = consts.tile([1, P], F32)
        nc.vector.memset(ones_row, 1.0)
        neg_ones_row = consts.tile([1, P], F32)
        nc.vector.memset(neg_ones_row, -1.0)
        ones_col = consts.tile([P, 1], F32)
        nc.vector.memset(ones_col, 1.0)

        # x[b] viewed so partition p holds rows p*NT .. p*NT+NT-1
        for b in range(BPC):
            xb = x_ap[b].rearrange("(p j) d -> p j d", p=P)  # [P, NT, D]

            # ---- stage 1: stream x (cast to fp16 in the DMA); fused
            #      multiply + free-axis reduce per tile ----
            E_cols = statp.tile([P, NT], F32, tag="E_cols")
            prod_b = prodp.tile([P, NT, D], FP16, tag="prod")
            for c in range(NCHUNK):
                xc = xpool.tile([P, CHUNK_TILES, D], FP16, tag="xc")
                nc.gpsimd.dma_start(
                    out=xc, in_=xb[:, c * CHUNK_TILES : (c + 1) * CHUNK_TILES, :]
                )
                for jj in range(CHUNK_TILES):
                    j = c * CHUNK_TILES + jj
                    nc.vector.scalar_tensor_tensor(
                        out=prod_b[:, j, :],
                        in0=xc[:, jj, :],
                        scalar=1.0,
                        in1=v_bcast,
                        op0=ALU.mult,
                        op1=ALU.mult,
                        accum_out=E_cols[:, j : j + 1],
                    )

            # ---- stage 2: softmax over the [P, NT] energy matrix ----
            m1 = statp.tile([P, 1], F32, tag="m1")
            nc.vector.tensor_reduce(m1, E_cols, axis=AX.X, op=ALU.max)
            mT_ps = ps_small.tile([1, P], F32, tag="ps_small")
            nc.tensor.transpose(mT_ps, m1, ident)
            mT = statp.tile([1, P], F32, tag="mT")
            nc.scalar.copy(mT, mT_ps)
            m = statp.tile([1, 1], F32, tag="m")
            nc.vector.tensor_reduce(m, mT, axis=AX.X, op=ALU.max)
            negm_ps = ps_small.tile([P, 1], F32, tag="ps_small")
            nc.tensor.matmul(negm_ps, lhsT=neg_ones_row, rhs=m, start=True, stop=True)
            negm = statp.tile([P, 1], F32, tag="negm")
            nc.scalar.copy(negm, negm_ps)
            P_cols = statp.tile([P, NT], FP16, tag="P_cols")
            s1 = statp.tile([P, 1], F32, tag="s1")
            nc.scalar.activation(
                P_cols, E_cols, ACTF.Exp, bias=negm, scale=1.0, accum_out=s1
            )

            # ---- stage 3: context accumulation (starts right after exp) ----
            ctx_ps = ps_ctx.tile([1, D], F32, tag="ctx_ps")
            for j in range(NT):
                nc.tensor.matmul(
                    ctx_ps,
                    lhsT=P_cols[:, j : j + 1],
                    rhs=prod_b[:, j, :],
                    start=(j == 0),
                    stop=(j == NT - 1),
                    skip_group_check=True,
                )

            # total sum across partitions, reciprocal, broadcast to partitions
            s_ps = ps_small.tile([1, 1], F32, tag="ps_small")
            nc.tensor.matmul(s_ps, lhsT=s1, rhs=ones_col, start=True, stop=True)
            s_sb = statp.tile([1, 1], F32, tag="s_sb")
            nc.scalar.copy(s_sb, s_ps)
            r = statp.tile([1, 1], F32, tag="r")
            nc.vector.reciprocal(r, s_sb)
            r_ps = ps_small.tile([P, 1], F32, tag="ps_small")
            nc.tensor.matmul(r_ps, lhsT=ones_row, rhs=r, start=True, stop=True)
            r_col = statp.tile([P, 1], F32, tag="r_col")
            nc.scalar.copy(r_col, r_ps)

            # weights output: w[b, p*NT + j] = P_cols[p, j] * r  (no transpose)
            w_sb = outp.tile([P, NT], F32, tag="w_sb")
            nc.scalar.activation(w_sb, P_cols, ACTF.Copy, bias=0.0, scale=r_col)
            nc.sync.dma_start(out=w_ap[b].rearrange("(p j) -> p j", p=P), in_=w_sb)

            # context epilogue: ctx = (ctx_ps * r) * (1/v)
            ctx_a = outp.tile([1, D], F32, tag="ctx_a")
            nc.scalar.activation(ctx_a, ctx_ps, ACTF.Copy, bias=0.0, scale=r)
            ctx_f = outp.tile([1, D], F32, tag="ctx_f")
            nc.gpsimd.tensor_mul(ctx_f, ctx_a, v_inv)
            nc.sync.dma_start(out=ctx_ap[b : b + 1, :], in_=ctx_f)


_CACHED_NC = None


def _get_nc():
    global _CACHED_NC
    if _CACHED_NC is not None:
        return _CACHED_NC
    nc = bacc.Bacc(
        "TRN2",
        target_bir_lowering=False,
        debug=False,
        enable_asserts=False,
        num_devices=NCORES,
    )
    x = nc.dram_tensor("x", [BPC, T, D], F32, kind="ExternalInput")
    vb = nc.dram_tensor("vb", [P, D], FP16, kind="ExternalInput")
    vinv = nc.dram_tensor("vinv", [1, D], F32, kind="ExternalInput")
    idm = nc.dram_tensor("idm", [P, P], F32, kind="ExternalInput")
    ctx_out = nc.dram_tensor("ctx", [BPC, D], F32, kind="ExternalOutput")
    w_out = nc.dram_tensor("w", [BPC, T], F32, kind="ExternalOutput")
    with tile.TileContext(nc) as tc:
        build_tile_kernel(tc, x.ap(), vb.ap(), vinv.ap(), idm.ap(), ctx_out.ap(), w_out.ap())
    nc.compile()
    _CACHED_NC = nc
    return nc


def _make_const_inputs(v):
    vrow = v[:, 0].astype(np.float32)                     # [D]
    vb = np.broadcast_to(vrow.astype(np.float16), (P, D)).copy()
    vinv = (1.0 / vrow)[None, :].copy()
    idm = np.eye(P, dtype=np.float32)
    return vb, vinv, idm


def _run(encoder_outputs, attn_weights_param, trace=False, **kw):
    nc = _get_nc()
    x = np.ascontiguousarray(np.asarray(encoder_outputs, dtype=np.float32))
    v = np.ascontiguousarray(np.asarray(attn_weights_param, dtype=np.float32))
    vb, vinv, idm = _make_const_inputs(v)
    in_maps = [
        {"x": x[c * BPC : (c + 1) * BPC], "vb": vb, "vinv": vinv, "idm": idm}
        for c in range(NCORES)
    ]
    res = bass_utils.run_bass_kernel_spmd(
        nc, in_maps, core_ids=list(range(NCORES)), trace=trace, **kw
    )
    context = np.concatenate([res.results[c]["ctx"] for c in range(NCORES)], axis=0)
    weights = np.concatenate([res.results[c]["w"] for c in range(NCORES)], axis=0)
    return (context, weights), res


def kernel(encoder_outputs, attn_weights_param):
    (context, weights), _ = _run(encoder_outputs, attn_weights_param, trace=False)
    return (context, weights)
